# revision 14
# baseline (speedup 1.0000x reference)
"""Trainium2 Bass kernel: multi-head attention (B=2, N=2048, DIM=768, H=12, Dh=64),
sharded (batch x head-group) across 8 NeuronCores. Self-contained.

Per-core shard (core = b*4 + g, g in 0..3, heads 3g..3g+2):
  - computes Q^T,K^T (features on partitions) and V (tokens on partitions) from x[b]^T
  - scores S^T[k,q] per head via row-tiled K=64 matmuls (2 concurrent per slot)
  - exp via ScalarE (table exp) + VectorE (custom cubic^4 approx), split per k-tile
  - O~^T/sums via augmented-V matmul (64 ones columns => sums broadcast on partitions 64:128)
  - normalize with approx reciprocal, project with proj_w rows, partial out [2048, 768] f32
Host: shards inputs, gathers partials: out[b] = sum_g partial + (b_v @ proj_w + proj_b).
"""

import sys

for _p in ("/opt/trn_rl_repo",):
    if _p not in sys.path:
        sys.path.append(_p)

import numpy as np
import ml_dtypes

import concourse.bass as bass
import concourse.mybir as mybir
import concourse.tile as tile
from concourse.bass_utils import run_bass_kernel_spmd

BF16 = mybir.dt.bfloat16
F32 = mybir.dt.float32
bf16 = ml_dtypes.bfloat16

B, N, DIM = 2, 2048, 768
H, Dh = 12, 64
G = 3  # heads per core
NCORES = 8
QC = 512  # query chunk (free dim of score matmuls)
NQC = N // QC
KT = 128  # key tile (partition dim of S^T)
NKT = N // KT

# exp split: which k-tiles go to the VectorE (custom poly) vs ScalarE (table exp).
# DVE k-tiles are singles; ACT k-tiles are grouped in pairs of 2 (one [128,1024] inst).
# NOTE: custom DVE ops fail to encode in this container's walrus ("ISA wrong
# length"), so all exp goes through ScalarE for now.
DVE_KTS = ()
ACT_PAIRS = tuple((2 * i, 2 * i + 1) for i in range(8))

# EXP4 constants: exp(x) ~ C0F^4 * ((1+A x)(1 + B x + CC x^2))^4 on |x| <= 2.75
EXP_A = 0.14770726095997042
EXP_B = 0.10315315610745052
EXP_CC = 0.017226206106509708
EXP_C0F = 0.9990441257079289
ACT_BIAS = -4.0 * float(np.log(EXP_C0F))  # ScalarE computes exp(x + bias) to match


# --------------------------------------------------------------------------
# workaround: this container's walrus accepts only ONE sync-wait per
# instruction ("Too many sync wait commands"). Split multi-wait sync_infos
# onto same-engine NoOps inserted right before the instruction.
def _patch_to_json():
    import orjson

    if getattr(bass.Bass, "_ant_json_patched", False):
        return
    orig = bass.Bass.to_json_bytes

    def to_json_bytes(self, *a, **kw):
        m = orjson.loads(orig(self, *a, **kw))

        def walk(o):
            if isinstance(o, dict):
                insts = o.get("instructions")
                if isinstance(insts, list) and insts and isinstance(insts[0], dict):
                    new = []
                    for inst in insts:
                        si = inst.get("sync_info")
                        waits = (si or {}).get("on_wait") or []
                        if len(waits) > 1:
                            for i, w in enumerate(waits[:-1]):
                                new.append(
                                    {
                                        "debug": inst.get("debug", 0),
                                        "engine": inst["engine"],
                                        "ins": [],
                                        "name": f"{inst['name']}-sw{i}",
                                        "opcode": "NoOp",
                                        "outs": [],
                                        "sync_info": {
                                            "on_update": [],
                                            "on_wait": [w],
                                        },
                                    }
                                )
                            si["on_wait"] = waits[-1:]
                        new.append(inst)
                    o["instructions"] = new
                for v in o.values():
                    walk(v)
            elif isinstance(o, list):
                for v in o:
                    walk(v)

        walk(m)
        return orjson.dumps(m)

    bass.Bass.to_json_bytes = to_json_bytes
    bass.Bass._ant_json_patched = True


# workaround: this container's walrus allows only 1 sync-wait on SP CTRL ops;
# Tile's kernel-tail drain piles every outstanding proc wait onto one Drain.
def _patch_tile_drain():
    from concourse.tile import TileContext, ScopedClock

    if getattr(TileContext, "_ant_drain_patched", False):
        return

    def _drain_and_barrier(self, tick_clock, wait_clock):
        nc = self.nc
        collector = nc.sync.nop(nofuse=True)
        wait_clock.add_sem_waits(
            collector.ins, ScopedClock({None: tick_clock.global_clock})
        )
        si = collector.ins.sync_info
        waits = list(si.on_wait) if si is not None else []
        if len(waits) > 1:
            si.on_wait = waits[:1]
            for w in waits[1:]:
                extra = nc.sync.nop(nofuse=True)
                extra.ins.sync_info = mybir.SyncInfo(on_wait=[w], on_update=[])
        nc.sync.drain()
        nc.all_engine_barrier()
        assert self.sems is not None
        popped = nc._tile_sem_poison_stack.pop()
        assert popped is self._sem_poison
        nc.clear_and_free_semaphores(list(self.sems.allocated().values()))
        nc.all_engine_barrier()

    TileContext._drain_and_barrier = _drain_and_barrier
    TileContext._ant_drain_patched = True


# --------------------------------------------------------------------------
# custom DVE ops: cubic ~ exp(x/4)/C0F (1 pass) and x -> x^4 (1 pass)
_EXP_OPS = {}


def _register_exp_ops():
    if _EXP_OPS:
        return _EXP_OPS
    from concourse import dve_ops
    from concourse.dve_ops import DveOp, OPS, _SUB_OPCODE_FOR_NAME
    from concourse.dve_spec import Spec, Src0, C0, C1, C2, One, sq, lower
    from concourse.dve_uop import DveOpSpec

    def make(name, spec):
        if name in _SUB_OPCODE_FOR_NAME:
            for op in OPS:
                if op.name == name:
                    return op
        row = max(_SUB_OPCODE_FOR_NAME.values()) + 1
        op = DveOp(name, spec, subdim=False, uops_sha={})
        OPS.append(op)
        _SUB_OPCODE_FOR_NAME[name] = row
        dve_ops.CUSTOM_DVE_SPECS[name] = spec
        for ver in ("v3", "v4"):
            uops = lower(spec, ver=ver)
            op.uops_sha[ver] = DveOpSpec(
                name=name, opcode=row, uops=uops, rd1_en=False
            ).sha(ver)
        return op

    cubic = make(
        "EXPC_ANT",
        Spec(
            body=(Src0 * C0 + One) * ((sq(Src0) * C2 + Src0 * C1) + One),
            reference=lambda in0, in1, s0, s1, imm2: (in0 * s0 + 1.0)
            * ((in0 * in0) * imm2 + in0 * s1 + 1.0),
        ),
    )
    pow4 = make(
        "POW4_ANT",
        Spec(
            body=sq(sq(Src0)),
            reference=lambda in0, in1, s0, s1, imm2: (in0 * in0) * (in0 * in0),
        ),
    )
    _EXP_OPS["cubic"] = cubic
    _EXP_OPS["pow4"] = pow4
    return _EXP_OPS


# --------------------------------------------------------------------------
def build_kernel():
    _patch_to_json()
    _patch_tile_drain()
    ops = _register_exp_ops() if DVE_KTS else None
    Exp = mybir.ActivationFunctionType.Exp
    Alu = mybir.AluOpType

    nc = bass.Bass(trn_type="TRN2")
    xT = nc.dram_tensor("xT", [DIM, N], BF16, kind="ExternalInput")
    wqk = nc.dram_tensor("wqk", [DIM, 384], BF16, kind="ExternalInput")
    bqk = nc.dram_tensor("bqk", [384], F32, kind="ExternalInput")
    wv = nc.dram_tensor("wv", [DIM, 192], BF16, kind="ExternalInput")
    wp = nc.dram_tensor("wp", [192, DIM], BF16, kind="ExternalInput")
    out = nc.dram_tensor("out", [N, DIM], F32, kind="ExternalOutput")

    KC = DIM // 128  # 6 contraction chunks

    with tile.TileContext(nc) as tc:
        with (
            tc.tile_pool(name="persist", bufs=1) as pp,
            tc.tile_pool(name="evict", bufs=3) as ep,
            tc.tile_pool(name="pt_act", bufs=4) as pta,
            tc.tile_pool(name="pt_dve", bufs=4) as ptd,
            tc.tile_pool(name="scratch", bufs=4) as sp,
            tc.tile_pool(name="osb", bufs=3) as op_,
            tc.tile_pool(name="ysb", bufs=3) as yp,
        ):
            # ---- persistent SBUF ----
            xT_sb = pp.tile([128, KC, N], BF16, tag="xT")
            wqk_sb = pp.tile([128, KC, 384], BF16, tag="wqk")
            wv_sb = pp.tile([128, KC, 192], BF16, tag="wv")
            wp_sb = pp.tile([128, 2, DIM], BF16, tag="wp")  # [:, 0]=rows 0:128, [0:64, 1]=rows 128:192
            bqk_sb = pp.tile([128, 3], F32, tag="bqk")
            actbias_sb = pp.tile([128, 1], F32, tag="actbias")
            nc.gpsimd.memset(actbias_sb[:], ACT_BIAS)
            qk_sb = pp.tile([128, 4, N], BF16, tag="qkT")  # mt: [Q0|Q1], [K0|K1], [Q2|K2], [K2d|Q2d]
            v_sb = pp.tile([128, NKT, 384], BF16, tag="vaug")  # per kt: [v0|1s, v1|1s, v2|1s] (64 each)

            nc.sync.dma_start(wqk_sb[:], wqk.rearrange("(o p) m -> p o m", p=128))
            nc.sync.dma_start(bqk_sb[:], bqk.rearrange("(m p) -> p m", p=128))
            for kc in range(KC):
                nc.sync.dma_start(xT_sb[:, kc, :], xT[128 * kc : 128 * kc + 128, :])
            nc.sync.dma_start(wv_sb[:], wv.rearrange("(o p) m -> p o m", p=128))
            nc.sync.dma_start(wp_sb[:, 0, :], wp[0:128, :])
            nc.sync.dma_start(wp_sb[0:64, 1, :], wp[128:192, :])
            # ones columns for the augmented-V sums trick
            nc.gpsimd.memset(v_sb[:], 1.0)

            # ---- phase 1a: Q^T / K^T ----
            with tc.tile_pool(name="ps1", bufs=3, space="PSUM") as ps1:
                for qc in range(NQC):
                    for mt in range(3):
                        ps = ps1.tile([128, QC], F32, tag="qk")
                        for kc in range(KC):
                            nc.tensor.matmul(
                                ps[:],
                                wqk_sb[:, kc, 128 * mt : 128 * mt + 128],
                                xT_sb[:, kc, QC * qc : QC * qc + QC],
                                start=(kc == 0),
                                stop=(kc == KC - 1),
                            )
                        # evict + bias + cast (bias indexed per output partition)
                        nc.vector.tensor_scalar(
                            qk_sb[:, mt, QC * qc : QC * qc + QC],
                            ps[:],
                            bqk_sb[:, mt : mt + 1],
                            None,
                            Alu.add,
                        )
                # mt3 = [K2dup | Q2dup] via partition-shifting SBUF DMAs
                nc.sync.dma_start(qk_sb[0:64, 3, :], qk_sb[64:128, 2, :])
                nc.sync.dma_start(qk_sb[64:128, 3, :], qk_sb[0:64, 2, :])

                # ---- phase 1b: V ----
                for kt in range(NKT):
                    ps = ps1.tile([128, 192], F32, tag="v")
                    for kc in range(KC):
                        nc.tensor.matmul(
                            ps[:],
                            xT_sb[:, kc, KT * kt : KT * kt + KT],
                            wv_sb[:, kc, :],
                            start=(kc == 0),
                            stop=(kc == KC - 1),
                        )
                    nc.vector.tensor_copy(
                        out=v_sb[:, kt, :].rearrange("p (h c) -> p h c", c=128)[:, :, 0:64],
                        in_=ps[:].rearrange("p (h c) -> p h c", c=64),
                    )

            # ---- phases 2+3 ----
            with (
                tc.tile_pool(name="ps_sa", bufs=2, space="PSUM") as ps_sa,
                tc.tile_pool(name="ps_acc", bufs=4, space="PSUM") as ps_acc,
            ):
                ps_y = ps_acc
                # score matmul operands per head: (lhsT tensor-slot, rhs tensor-slot, partition half)
                # heads 0/1: K from mt1, Q from mt0, halves 0/1. head 2: alternate halves by kt.
                def s_operands(h, kt):
                    if h < 2:
                        po = 64 * h
                        return (1, po), (0, po)
                    return ((3, 0) if kt % 2 == 0 else (2, 64)), ((2, 0) if kt % 2 == 0 else (3, 64))

                for qc in range(NQC):
                    o_a = op_.tile([128, QC], BF16, tag="oa", name="oa")
                    o_b = op_.tile([64, QC], BF16, tag="ob", name="ob")
                    for heads in ((0, 1), (2,)):
                        o_ps = {
                            h: ps_acc.tile([128, QC], F32, tag="acc", name="acc")
                            for h in heads
                        }
                        for k0, k1 in ACT_PAIRS:
                            s2 = {}
                            for h in heads:
                                s2[h] = ps_sa.tile([128, 2 * QC], F32, tag="sa", name="sa")
                            # interleave heads so T0/T8 row-tiles run concurrently
                            for kk, off in ((k0, 0), (k1, QC)):
                                for h in heads:
                                    (lm, lp), (rm, rp) = s_operands(h, kk)
                                    nc.tensor.matmul(
                                        s2[h][:, off : off + QC],
                                        qk_sb[lp : lp + 64, lm, KT * kk : KT * kk + KT],
                                        qk_sb[rp : rp + 64, rm, QC * qc : QC * qc + QC],
                                        start=True,
                                        stop=True,
                                        tile_position=(lp, 0),
                                    )
                            for h in heads:
                                pt = pta.tile([128, 2 * QC], BF16, tag="pta", name="pta")
                                nc.scalar.activation(pt[:], s2[h][:], Exp, bias=actbias_sb[:])
                                for kk, off in ((k0, 0), (k1, QC)):
                                    nc.tensor.matmul(
                                        o_ps[h][:],
                                        v_sb[:, kk, 128 * h : 128 * h + 128],
                                        pt[:, off : off + QC],
                                        start=(kk == 0),
                                        stop=(kk == NKT - 1),
                                    )
                        # normalize: rows 64:128 hold sums (broadcast via ones cols)
                        for h in heads:
                            rec = sp.tile([64, QC], F32, tag="recip")
                            nc.vector.reciprocal(out=rec[:], in_=o_ps[h][64:128, :])
                            dst = o_a[64 * h : 64 * h + 64, :] if h < 2 else o_b[:]
                            nc.vector.tensor_tensor(dst, o_ps[h][0:64, :], rec[:], Alu.mult)

                    # ---- phase 3: projection for this query chunk ----
                    for qt in range(QC // 128):
                        ys = yp.tile([128, DIM], F32, tag="y")
                        for nc2 in range(2):
                            nsl = slice(384 * nc2, 384 * nc2 + 384)
                            yps = ps_y.tile([128, QC], F32, tag="acc", name="yps")[:, 0:384]
                            nc.tensor.matmul(
                                yps[:],
                                o_a[:, 128 * qt : 128 * qt + 128],
                                wp_sb[:, 0, nsl],
                                start=True,
                                stop=False,
                            )
                            nc.tensor.matmul(
                                yps[:],
                                o_b[:, 128 * qt : 128 * qt + 128],
                                wp_sb[0:64, 1, nsl],
                                start=False,
                                stop=True,
                            )
                            nc.vector.tensor_copy(out=ys[:, nsl], in_=yps[:])
                        nc.sync.dma_start(
                            out[QC * qc + 128 * qt : QC * qc + 128 * qt + 128, :], ys[:]
                        )
    return nc


_NC_CACHE = {}


def _get_nc():
    if "nc" not in _NC_CACHE:
        _NC_CACHE["nc"] = build_kernel()
    return _NC_CACHE["nc"]


def kernel(x, qkv_w, qkv_b, proj_w, proj_b):
    x = np.asarray(x, np.float32)
    qkv_w = np.asarray(qkv_w, np.float32)
    qkv_b = np.asarray(qkv_b, np.float32)
    proj_w = np.asarray(proj_w, np.float32)
    proj_b = np.asarray(proj_b, np.float32)

    wr = qkv_w.reshape(DIM, 3, H, Dh)
    br = qkv_b.reshape(3, H, Dh)
    scale = Dh ** -0.5

    in_maps = []
    for core in range(NCORES):
        b, g = divmod(core, 4)
        hs = slice(G * g, G * g + G)
        wq = wr[:, 0, hs, :].reshape(DIM, G * Dh) * scale  # fold softmax scale into Q
        wk = wr[:, 1, hs, :].reshape(DIM, G * Dh)
        wvm = wr[:, 2, hs, :].reshape(DIM, G * Dh)
        bq = br[0, hs].reshape(G * Dh) * scale
        bk = br[1, hs].reshape(G * Dh)
        # column order: mt0=[Q0|Q1], mt1=[K0|K1], mt2=[Q2|K2] (64 cols per head)
        wqk_c = np.concatenate(
            [wq[:, 0:128], wk[:, 0:128], wq[:, 128:192], wk[:, 128:192]], axis=1
        )
        bqk_c = np.concatenate([bq[0:128], bk[0:128], bq[128:192], bk[128:192]])
        in_maps.append(
            {
                "xT": np.ascontiguousarray(x[b].T).astype(bf16),
                "wqk": np.ascontiguousarray(wqk_c).astype(bf16),
                "bqk": np.ascontiguousarray(bqk_c),
                "wv": np.ascontiguousarray(wvm).astype(bf16),
                "wp": np.ascontiguousarray(proj_w[64 * G * g : 64 * G * (g + 1), :]).astype(bf16),
            }
        )

    nc = _get_nc()
    res = run_bass_kernel_spmd(nc, in_maps, core_ids=list(range(NCORES)))
    _NC_CACHE["last_result"] = res

    bias_row = (br[2].reshape(DIM).astype(np.float64) @ proj_w.astype(np.float64)
                + proj_b.astype(np.float64)).astype(np.float32)
    out = np.zeros((B, N, DIM), np.float32)
    for b in range(B):
        acc = np.zeros((N, DIM), np.float64)
        for g in range(4):
            acc += res.results[4 * b + g]["out"].astype(np.float64)
        out[b] = acc.astype(np.float32) + bias_row
    return out


# revision 15
# speedup vs baseline: 1.0989x; 1.0989x over previous
"""Trainium2 Bass kernel: multi-head attention (B=2, N=2048, DIM=768, H=12, Dh=64),
sharded (batch x head-group) across 8 NeuronCores. Self-contained.

Per-core shard (core = b*4 + g, g in 0..3, heads 3g..3g+2):
  - computes Q^T,K^T (features on partitions) and V (tokens on partitions) from x[b]^T
  - scores S^T[k,q] per head via row-tiled K=64 matmuls (2 concurrent per slot)
  - exp via ScalarE (table exp) + VectorE (custom cubic^4 approx), split per k-tile
  - O~^T/sums via augmented-V matmul (64 ones columns => sums broadcast on partitions 64:128)
  - normalize with approx reciprocal, project with proj_w rows, partial out [2048, 768] f32
Host: shards inputs, gathers partials: out[b] = sum_g partial + (b_v @ proj_w + proj_b).
"""

import sys

for _p in ("/opt/trn_rl_repo",):
    if _p not in sys.path:
        sys.path.append(_p)

import numpy as np
import ml_dtypes

import concourse.bass as bass
import concourse.mybir as mybir
import concourse.tile as tile
from concourse.bass_utils import run_bass_kernel_spmd

BF16 = mybir.dt.bfloat16
F32 = mybir.dt.float32
bf16 = ml_dtypes.bfloat16

B, N, DIM = 2, 2048, 768
H, Dh = 12, 64
G = 3  # heads per core
NCORES = 8
QC = 512  # query chunk (free dim of score matmuls)
NQC = N // QC
KT = 128  # key tile (partition dim of S^T)
NKT = N // KT

# exp split: which k-tiles go to the VectorE (custom poly) vs ScalarE (table exp).
# DVE k-tiles are singles; ACT k-tiles are grouped in pairs of 2 (one [128,1024] inst).
# NOTE: custom DVE ops fail to encode in this container's walrus ("ISA wrong
# length"), so all exp goes through ScalarE for now.
DVE_KTS = ()
ACT_PAIRS = tuple((2 * i, 2 * i + 1) for i in range(8))

# EXP4 constants: exp(x) ~ C0F^4 * ((1+A x)(1 + B x + CC x^2))^4 on |x| <= 2.75
EXP_A = 0.14770726095997042
EXP_B = 0.10315315610745052
EXP_CC = 0.017226206106509708
EXP_C0F = 0.9990441257079289
ACT_BIAS = -4.0 * float(np.log(EXP_C0F))  # ScalarE computes exp(x + bias) to match


# --------------------------------------------------------------------------
# workaround: this container's walrus accepts only ONE sync-wait per
# instruction ("Too many sync wait commands"). Split multi-wait sync_infos
# onto same-engine NoOps inserted right before the instruction.
def _patch_to_json():
    import orjson

    if getattr(bass.Bass, "_ant_json_patched", False):
        return
    orig = bass.Bass.to_json_bytes

    def to_json_bytes(self, *a, **kw):
        m = orjson.loads(orig(self, *a, **kw))

        def walk(o):
            if isinstance(o, dict):
                insts = o.get("instructions")
                if isinstance(insts, list) and insts and isinstance(insts[0], dict):
                    new = []
                    for inst in insts:
                        si = inst.get("sync_info")
                        waits = (si or {}).get("on_wait") or []
                        if len(waits) > 1:
                            for i, w in enumerate(waits[:-1]):
                                new.append(
                                    {
                                        "debug": inst.get("debug", 0),
                                        "engine": inst["engine"],
                                        "ins": [],
                                        "name": f"{inst['name']}-sw{i}",
                                        "opcode": "NoOp",
                                        "outs": [],
                                        "sync_info": {
                                            "on_update": [],
                                            "on_wait": [w],
                                        },
                                    }
                                )
                            si["on_wait"] = waits[-1:]
                        new.append(inst)
                    o["instructions"] = new
                for v in o.values():
                    walk(v)
            elif isinstance(o, list):
                for v in o:
                    walk(v)

        walk(m)
        return orjson.dumps(m)

    bass.Bass.to_json_bytes = to_json_bytes
    bass.Bass._ant_json_patched = True


# workaround: this container's walrus allows only 1 sync-wait on SP CTRL ops;
# Tile's kernel-tail drain piles every outstanding proc wait onto one Drain.
def _patch_tile_drain():
    from concourse.tile import TileContext, ScopedClock

    if getattr(TileContext, "_ant_drain_patched", False):
        return

    def _drain_and_barrier(self, tick_clock, wait_clock):
        nc = self.nc
        collector = nc.sync.nop(nofuse=True)
        wait_clock.add_sem_waits(
            collector.ins, ScopedClock({None: tick_clock.global_clock})
        )
        si = collector.ins.sync_info
        waits = list(si.on_wait) if si is not None else []
        if len(waits) > 1:
            si.on_wait = waits[:1]
            for w in waits[1:]:
                extra = nc.sync.nop(nofuse=True)
                extra.ins.sync_info = mybir.SyncInfo(on_wait=[w], on_update=[])
        nc.sync.drain()
        nc.all_engine_barrier()
        assert self.sems is not None
        popped = nc._tile_sem_poison_stack.pop()
        assert popped is self._sem_poison
        nc.clear_and_free_semaphores(list(self.sems.allocated().values()))
        nc.all_engine_barrier()

    TileContext._drain_and_barrier = _drain_and_barrier
    TileContext._ant_drain_patched = True


# --------------------------------------------------------------------------
# custom DVE ops: cubic ~ exp(x/4)/C0F (1 pass) and x -> x^4 (1 pass)
_EXP_OPS = {}


def _register_exp_ops():
    if _EXP_OPS:
        return _EXP_OPS
    from concourse import dve_ops
    from concourse.dve_ops import DveOp, OPS, _SUB_OPCODE_FOR_NAME
    from concourse.dve_spec import Spec, Src0, C0, C1, C2, One, sq, lower
    from concourse.dve_uop import DveOpSpec

    def make(name, spec):
        if name in _SUB_OPCODE_FOR_NAME:
            for op in OPS:
                if op.name == name:
                    return op
        row = max(_SUB_OPCODE_FOR_NAME.values()) + 1
        op = DveOp(name, spec, subdim=False, uops_sha={})
        OPS.append(op)
        _SUB_OPCODE_FOR_NAME[name] = row
        dve_ops.CUSTOM_DVE_SPECS[name] = spec
        for ver in ("v3", "v4"):
            uops = lower(spec, ver=ver)
            op.uops_sha[ver] = DveOpSpec(
                name=name, opcode=row, uops=uops, rd1_en=False
            ).sha(ver)
        return op

    cubic = make(
        "EXPC_ANT",
        Spec(
            body=(Src0 * C0 + One) * ((sq(Src0) * C2 + Src0 * C1) + One),
            reference=lambda in0, in1, s0, s1, imm2: (in0 * s0 + 1.0)
            * ((in0 * in0) * imm2 + in0 * s1 + 1.0),
        ),
    )
    pow4 = make(
        "POW4_ANT",
        Spec(
            body=sq(sq(Src0)),
            reference=lambda in0, in1, s0, s1, imm2: (in0 * in0) * (in0 * in0),
        ),
    )
    _EXP_OPS["cubic"] = cubic
    _EXP_OPS["pow4"] = pow4
    return _EXP_OPS


# --------------------------------------------------------------------------
def build_kernel():
    _patch_to_json()
    _patch_tile_drain()
    ops = _register_exp_ops() if DVE_KTS else None
    Exp = mybir.ActivationFunctionType.Exp
    Alu = mybir.AluOpType

    nc = bass.Bass(trn_type="TRN2")
    xT = nc.dram_tensor("xT", [DIM, N], BF16, kind="ExternalInput")
    wqk = nc.dram_tensor("wqk", [DIM, 384], BF16, kind="ExternalInput")
    bqk = nc.dram_tensor("bqk", [384], F32, kind="ExternalInput")
    wv = nc.dram_tensor("wv", [DIM, 192], BF16, kind="ExternalInput")
    wp = nc.dram_tensor("wp", [192, DIM], BF16, kind="ExternalInput")
    out = nc.dram_tensor("out", [N, DIM], F32, kind="ExternalOutput")

    KC = DIM // 128  # 6 contraction chunks

    with tile.TileContext(nc) as tc:
        with (
            tc.tile_pool(name="persist", bufs=1) as pp,
            tc.tile_pool(name="evict", bufs=3) as ep,
            tc.tile_pool(name="pt_act", bufs=4) as pta,
            tc.tile_pool(name="pt_dve", bufs=4) as ptd,
            tc.tile_pool(name="scratch", bufs=4) as sp,
            tc.tile_pool(name="osb", bufs=3) as op_,
            tc.tile_pool(name="ysb", bufs=3) as yp,
        ):
            # ---- persistent SBUF ----
            xT_sb = pp.tile([128, KC, N], BF16, tag="xT")
            wqk_sb = pp.tile([128, KC, 384], BF16, tag="wqk")
            wv_sb = pp.tile([128, KC, 192], BF16, tag="wv")
            wp_sb = pp.tile([128, 2, DIM], BF16, tag="wp")  # [:, 0]=rows 0:128, [0:64, 1]=rows 128:192
            bqk_sb = pp.tile([128, 3], F32, tag="bqk")
            actbias_sb = pp.tile([128, 1], F32, tag="actbias")
            nc.gpsimd.memset(actbias_sb[:], ACT_BIAS)
            qk_sb = pp.tile([128, 4, N], BF16, tag="qkT")  # mt: [Q0|Q1], [K0|K1], [Q2|K2], [K2d|Q2d]
            v_sb = pp.tile([128, NKT, 384], BF16, tag="vaug")  # per kt: [v0|1s, v1|1s, v2|1s] (64 each)

            nc.sync.dma_start(wqk_sb[:], wqk.rearrange("(o p) m -> p o m", p=128))
            nc.sync.dma_start(bqk_sb[:], bqk.rearrange("(m p) -> p m", p=128))
            for kc in range(KC):
                nc.sync.dma_start(xT_sb[:, kc, :], xT[128 * kc : 128 * kc + 128, :])
            nc.sync.dma_start(wv_sb[:], wv.rearrange("(o p) m -> p o m", p=128))
            nc.sync.dma_start(wp_sb[:, 0, :], wp[0:128, :])
            nc.sync.dma_start(wp_sb[0:64, 1, :], wp[128:192, :])
            # ones columns for the augmented-V sums trick
            nc.gpsimd.memset(v_sb[:], 1.0)

            # ---- phase 1a: Q^T / K^T ----
            with tc.tile_pool(name="ps1", bufs=3, space="PSUM") as ps1:
                for qc in range(NQC):
                    for mt in range(3):
                        ps = ps1.tile([128, QC], F32, tag="qk")
                        for kc in range(KC):
                            nc.tensor.matmul(
                                ps[:],
                                wqk_sb[:, kc, 128 * mt : 128 * mt + 128],
                                xT_sb[:, kc, QC * qc : QC * qc + QC],
                                start=(kc == 0),
                                stop=(kc == KC - 1),
                            )
                        # evict + bias + cast (bias indexed per output partition)
                        nc.vector.tensor_scalar(
                            qk_sb[:, mt, QC * qc : QC * qc + QC],
                            ps[:],
                            bqk_sb[:, mt : mt + 1],
                            None,
                            Alu.add,
                        )
                # mt3 = [K2dup | Q2dup] via partition-shifting SBUF DMAs
                nc.sync.dma_start(qk_sb[0:64, 3, :], qk_sb[64:128, 2, :])
                nc.sync.dma_start(qk_sb[64:128, 3, :], qk_sb[0:64, 2, :])

                # ---- phase 1b: V ----
                for kt in range(NKT):
                    ps = ps1.tile([128, 192], F32, tag="v")
                    for kc in range(KC):
                        nc.tensor.matmul(
                            ps[:],
                            xT_sb[:, kc, KT * kt : KT * kt + KT],
                            wv_sb[:, kc, :],
                            start=(kc == 0),
                            stop=(kc == KC - 1),
                        )
                    nc.vector.tensor_copy(
                        out=v_sb[:, kt, :].rearrange("p (h c) -> p h c", c=128)[:, :, 0:64],
                        in_=ps[:].rearrange("p (h c) -> p h c", c=64),
                    )

            # ---- phases 2+3 ----
            # One [128, 1024] f32 PSUM tile per k-tile step holds BOTH heads of a
            # pair ([0:512] first head, [512:1024] second), so the two K=64
            # row-tiled score matmuls run concurrently and one ScalarE exp
            # covers both. 3-deep ring => scores of step k+1 run while step k
            # is in exp and step k-1 in PV.
            with (
                tc.tile_pool(name="ps_sa", bufs=3, space="PSUM") as ps_sa,
                tc.tile_pool(name="ps_acc", bufs=2, space="PSUM") as ps_acc,
            ):
                # score matmul operands per head: (lhsT tensor-slot, rhs tensor-slot, partition half)
                # heads 0/1: K from mt1, Q from mt0, halves 0/1. head 2: alternate halves by kt.
                def s_operands(h, kt):
                    if h < 2:
                        po = 64 * h
                        return (1, po), (0, po)
                    return ((3, 0) if kt % 2 == 0 else (2, 64)), ((2, 0) if kt % 2 == 0 else (3, 64))

                def s_mm(dst, h, kt, qc):
                    (lm, lp), (rm, rp) = s_operands(h, kt)
                    nc.tensor.matmul(
                        dst,
                        qk_sb[lp : lp + 64, lm, KT * kt : KT * kt + KT],
                        qk_sb[rp : rp + 64, rm, QC * qc : QC * qc + QC],
                        start=True,
                        stop=True,
                        tile_position=(lp, 0),
                    )

                for qc in range(NQC):
                    o_a = op_.tile([128, QC], BF16, tag="oa", name="oa")
                    o_b = op_.tile([64, QC], BF16, tag="ob", name="ob")
                    for heads in ((0, 1), (2,)):
                        o_ps = {
                            h: ps_acc.tile([128, QC], F32, tag="acc", name="acc")
                            for h in heads
                        }
                        # steps: pair -> one kt per step; head2 -> two kt per step
                        steps = (
                            [((heads[0], kt, 0), (heads[1], kt, QC)) for kt in range(NKT)]
                            if len(heads) == 2
                            else [((2, 2 * i, 0), (2, 2 * i + 1, QC)) for i in range(NKT // 2)]
                        )
                        for step in steps:
                            s2 = ps_sa.tile([128, 2 * QC], F32, tag="sa", name="sa")
                            for h, kt, off in step:
                                s_mm(s2[:, off : off + QC], h, kt, qc)
                            pt = pta.tile([128, 2 * QC], BF16, tag="pta", name="pta")
                            nc.scalar.activation(pt[:], s2[:], Exp, bias=actbias_sb[:])
                            for h, kt, off in step:
                                nc.tensor.matmul(
                                    o_ps[h][:],
                                    v_sb[:, kt, 128 * h : 128 * h + 128],
                                    pt[:, off : off + QC],
                                    start=(kt == 0),
                                    stop=(kt == NKT - 1),
                                )
                        # evacuate accumulators to SBUF fast (frees the PSUM bank),
                        # then normalize from the SBUF copy off the critical path.
                        for h in heads:
                            oc = sp.tile([128, QC], F32, tag="ocopy", name="ocopy")
                            nc.vector.tensor_copy(out=oc[:], in_=o_ps[h][:])
                            rec = sp.tile([64, QC], F32, tag="recip", name="recip")
                            nc.vector.reciprocal(out=rec[:], in_=oc[64:128, :])
                            dst = o_a[64 * h : 64 * h + 64, :] if h < 2 else o_b[:]
                            nc.vector.tensor_tensor(dst, oc[0:64, :], rec[:], Alu.mult)

                    # ---- phase 3: projection for this query chunk ----
                    for qt in range(QC // 128):
                        ys = yp.tile([128, DIM], F32, tag="y")
                        for nc2 in range(2):
                            nsl = slice(384 * nc2, 384 * nc2 + 384)
                            yps = ps_sa.tile([128, 2 * QC], F32, tag="sa", name="yps")[:, 0:384]
                            nc.tensor.matmul(
                                yps[:],
                                o_a[:, 128 * qt : 128 * qt + 128],
                                wp_sb[:, 0, nsl],
                                start=True,
                                stop=False,
                            )
                            nc.tensor.matmul(
                                yps[:],
                                o_b[:, 128 * qt : 128 * qt + 128],
                                wp_sb[0:64, 1, nsl],
                                start=False,
                                stop=True,
                            )
                            nc.vector.tensor_copy(out=ys[:, nsl], in_=yps[:])
                        nc.sync.dma_start(
                            out[QC * qc + 128 * qt : QC * qc + 128 * qt + 128, :], ys[:]
                        )
    return nc


_NC_CACHE = {}


def _get_nc():
    if "nc" not in _NC_CACHE:
        _NC_CACHE["nc"] = build_kernel()
    return _NC_CACHE["nc"]


def kernel(x, qkv_w, qkv_b, proj_w, proj_b):
    x = np.asarray(x, np.float32)
    qkv_w = np.asarray(qkv_w, np.float32)
    qkv_b = np.asarray(qkv_b, np.float32)
    proj_w = np.asarray(proj_w, np.float32)
    proj_b = np.asarray(proj_b, np.float32)

    wr = qkv_w.reshape(DIM, 3, H, Dh)
    br = qkv_b.reshape(3, H, Dh)
    scale = Dh ** -0.5

    in_maps = []
    for core in range(NCORES):
        b, g = divmod(core, 4)
        hs = slice(G * g, G * g + G)
        wq = wr[:, 0, hs, :].reshape(DIM, G * Dh) * scale  # fold softmax scale into Q
        wk = wr[:, 1, hs, :].reshape(DIM, G * Dh)
        wvm = wr[:, 2, hs, :].reshape(DIM, G * Dh)
        bq = br[0, hs].reshape(G * Dh) * scale
        bk = br[1, hs].reshape(G * Dh)
        # column order: mt0=[Q0|Q1], mt1=[K0|K1], mt2=[Q2|K2] (64 cols per head)
        wqk_c = np.concatenate(
            [wq[:, 0:128], wk[:, 0:128], wq[:, 128:192], wk[:, 128:192]], axis=1
        )
        bqk_c = np.concatenate([bq[0:128], bk[0:128], bq[128:192], bk[128:192]])
        in_maps.append(
            {
                "xT": np.ascontiguousarray(x[b].T).astype(bf16),
                "wqk": np.ascontiguousarray(wqk_c).astype(bf16),
                "bqk": np.ascontiguousarray(bqk_c),
                "wv": np.ascontiguousarray(wvm).astype(bf16),
                "wp": np.ascontiguousarray(proj_w[64 * G * g : 64 * G * (g + 1), :]).astype(bf16),
            }
        )

    nc = _get_nc()
    res = run_bass_kernel_spmd(nc, in_maps, core_ids=list(range(NCORES)))
    _NC_CACHE["last_result"] = res

    bias_row = (br[2].reshape(DIM).astype(np.float64) @ proj_w.astype(np.float64)
                + proj_b.astype(np.float64)).astype(np.float32)
    out = np.zeros((B, N, DIM), np.float32)
    for b in range(B):
        acc = np.zeros((N, DIM), np.float64)
        for g in range(4):
            acc += res.results[4 * b + g]["out"].astype(np.float64)
        out[b] = acc.astype(np.float32) + bias_row
    return out


# revision 17
# speedup vs baseline: 1.2141x; 1.1048x over previous
"""Trainium2 Bass kernel: multi-head attention (B=2, N=2048, DIM=768, H=12, Dh=64),
sharded (batch x head-group) across 8 NeuronCores. Self-contained.

Per-core shard (core = b*4 + g, g in 0..3, heads 3g..3g+2):
  - computes Q^T,K^T (features on partitions) and V (tokens on partitions) from x[b]^T
  - scores S^T[k,q] per head via row-tiled K=64 matmuls (2 concurrent per slot)
  - exp via ScalarE (table exp) + VectorE (custom cubic^4 approx), split per k-tile
  - O~^T/sums via augmented-V matmul (64 ones columns => sums broadcast on partitions 64:128)
  - normalize with approx reciprocal, project with proj_w rows, partial out [2048, 768] f32
Host: shards inputs, gathers partials: out[b] = sum_g partial + (b_v @ proj_w + proj_b).
"""

import sys

for _p in ("/opt/trn_rl_repo",):
    if _p not in sys.path:
        sys.path.append(_p)

import numpy as np
import ml_dtypes

import concourse.bass as bass
import concourse.mybir as mybir
import concourse.tile as tile
from concourse.bass_utils import run_bass_kernel_spmd

BF16 = mybir.dt.bfloat16
F32 = mybir.dt.float32
bf16 = ml_dtypes.bfloat16

B, N, DIM = 2, 2048, 768
H, Dh = 12, 64
G = 3  # heads per core
NCORES = 8
QC = 512  # query chunk (free dim of score matmuls)
NQC = N // QC
KT = 128  # key tile (partition dim of S^T)
NKT = N // KT

# exp split: which k-tiles go to the VectorE (custom poly) vs ScalarE (table exp).
# DVE k-tiles are singles; ACT k-tiles are grouped in pairs of 2 (one [128,1024] inst).
# NOTE: custom DVE ops fail to encode in this container's walrus ("ISA wrong
# length"), so all exp goes through ScalarE for now.
DVE_KTS = ()
ACT_PAIRS = tuple((2 * i, 2 * i + 1) for i in range(8))

# EXP4 constants: exp(x) ~ C0F^4 * ((1+A x)(1 + B x + CC x^2))^4 on |x| <= 2.75
EXP_A = 0.14770726095997042
EXP_B = 0.10315315610745052
EXP_CC = 0.017226206106509708
EXP_C0F = 0.9990441257079289
ACT_BIAS = -4.0 * float(np.log(EXP_C0F))  # ScalarE computes exp(x + bias) to match


# --------------------------------------------------------------------------
# workaround: this container's walrus accepts only ONE sync-wait per
# instruction ("Too many sync wait commands"). Split multi-wait sync_infos
# onto same-engine NoOps inserted right before the instruction.
def _patch_to_json():
    import orjson

    if getattr(bass.Bass, "_ant_json_patched", False):
        return
    orig = bass.Bass.to_json_bytes

    def to_json_bytes(self, *a, **kw):
        m = orjson.loads(orig(self, *a, **kw))

        def walk(o):
            if isinstance(o, dict):
                insts = o.get("instructions")
                if isinstance(insts, list) and insts and isinstance(insts[0], dict):
                    new = []
                    for inst in insts:
                        si = inst.get("sync_info")
                        waits = (si or {}).get("on_wait") or []
                        if len(waits) > 1:
                            for i, w in enumerate(waits[:-1]):
                                new.append(
                                    {
                                        "debug": inst.get("debug", 0),
                                        "engine": inst["engine"],
                                        "ins": [],
                                        "name": f"{inst['name']}-sw{i}",
                                        "opcode": "NoOp",
                                        "outs": [],
                                        "sync_info": {
                                            "on_update": [],
                                            "on_wait": [w],
                                        },
                                    }
                                )
                            si["on_wait"] = waits[-1:]
                        new.append(inst)
                    o["instructions"] = new
                for v in o.values():
                    walk(v)
            elif isinstance(o, list):
                for v in o:
                    walk(v)

        walk(m)
        return orjson.dumps(m)

    bass.Bass.to_json_bytes = to_json_bytes
    bass.Bass._ant_json_patched = True


# workaround: this container's walrus allows only 1 sync-wait on SP CTRL ops;
# Tile's kernel-tail drain piles every outstanding proc wait onto one Drain.
def _patch_tile_drain():
    from concourse.tile import TileContext, ScopedClock

    if getattr(TileContext, "_ant_drain_patched", False):
        return

    def _drain_and_barrier(self, tick_clock, wait_clock):
        nc = self.nc
        collector = nc.sync.nop(nofuse=True)
        wait_clock.add_sem_waits(
            collector.ins, ScopedClock({None: tick_clock.global_clock})
        )
        si = collector.ins.sync_info
        waits = list(si.on_wait) if si is not None else []
        if len(waits) > 1:
            si.on_wait = waits[:1]
            for w in waits[1:]:
                extra = nc.sync.nop(nofuse=True)
                extra.ins.sync_info = mybir.SyncInfo(on_wait=[w], on_update=[])
        nc.sync.drain()
        nc.all_engine_barrier()
        assert self.sems is not None
        popped = nc._tile_sem_poison_stack.pop()
        assert popped is self._sem_poison
        nc.clear_and_free_semaphores(list(self.sems.allocated().values()))
        nc.all_engine_barrier()

    TileContext._drain_and_barrier = _drain_and_barrier
    TileContext._ant_drain_patched = True


# --------------------------------------------------------------------------
# custom DVE ops: cubic ~ exp(x/4)/C0F (1 pass) and x -> x^4 (1 pass)
_EXP_OPS = {}


def _register_exp_ops():
    if _EXP_OPS:
        return _EXP_OPS
    from concourse import dve_ops
    from concourse.dve_ops import DveOp, OPS, _SUB_OPCODE_FOR_NAME
    from concourse.dve_spec import Spec, Src0, C0, C1, C2, One, sq, lower
    from concourse.dve_uop import DveOpSpec

    def make(name, spec):
        if name in _SUB_OPCODE_FOR_NAME:
            for op in OPS:
                if op.name == name:
                    return op
        row = max(_SUB_OPCODE_FOR_NAME.values()) + 1
        op = DveOp(name, spec, subdim=False, uops_sha={})
        OPS.append(op)
        _SUB_OPCODE_FOR_NAME[name] = row
        dve_ops.CUSTOM_DVE_SPECS[name] = spec
        for ver in ("v3", "v4"):
            uops = lower(spec, ver=ver)
            op.uops_sha[ver] = DveOpSpec(
                name=name, opcode=row, uops=uops, rd1_en=False
            ).sha(ver)
        return op

    cubic = make(
        "EXPC_ANT",
        Spec(
            body=(Src0 * C0 + One) * ((sq(Src0) * C2 + Src0 * C1) + One),
            reference=lambda in0, in1, s0, s1, imm2: (in0 * s0 + 1.0)
            * ((in0 * in0) * imm2 + in0 * s1 + 1.0),
        ),
    )
    pow4 = make(
        "POW4_ANT",
        Spec(
            body=sq(sq(Src0)),
            reference=lambda in0, in1, s0, s1, imm2: (in0 * in0) * (in0 * in0),
        ),
    )
    _EXP_OPS["cubic"] = cubic
    _EXP_OPS["pow4"] = pow4
    return _EXP_OPS


# --------------------------------------------------------------------------
def build_kernel():
    _patch_to_json()
    _patch_tile_drain()
    Exp = mybir.ActivationFunctionType.Exp
    Alu = mybir.AluOpType

    nc = bass.Bass(trn_type="TRN2")
    xT = nc.dram_tensor("xT", [DIM, N], BF16, kind="ExternalInput")
    wqk = nc.dram_tensor("wqk", [DIM, 384], BF16, kind="ExternalInput")
    bqk = nc.dram_tensor("bqk", [384], F32, kind="ExternalInput")
    wv = nc.dram_tensor("wv", [DIM, 192], BF16, kind="ExternalInput")
    wp = nc.dram_tensor("wp", [192, DIM], BF16, kind="ExternalInput")
    out = nc.dram_tensor("out", [N, DIM], F32, kind="ExternalOutput")

    KC = DIM // 128  # 6 contraction chunks

    with tile.TileContext(nc) as tc:
        with (
            tc.tile_pool(name="persist", bufs=1) as pp,
            tc.tile_pool(name="pt_act", bufs=4) as pta,
            tc.tile_pool(name="scratch", bufs=4) as sp,
            tc.tile_pool(name="osb", bufs=3) as op_,
            tc.tile_pool(name="ysb", bufs=3) as yp,
            tc.tile_pool(name="ps", bufs=3, space="PSUM") as ps,
            tc.tile_pool(name="ps_acc", bufs=2, space="PSUM") as ps_acc,
        ):
            # ---- persistent SBUF ----
            xT_sb = pp.tile([128, KC, N], BF16, tag="xT")
            wqk_sb = pp.tile([128, KC, 384], BF16, tag="wqk")
            wv_sb = pp.tile([128, KC, 192], BF16, tag="wv")
            wp_sb = pp.tile([128, 2, DIM], BF16, tag="wp")
            bqk_sb = pp.tile([128, 3], F32, tag="bqk")
            actbias_sb = pp.tile([128, 1], F32, tag="actbias")
            warm_sb = pp.tile([128, 8], BF16, tag="warm")
            qk_sb = pp.tile([128, 4, N], BF16, tag="qkT")  # mt: [Q0|Q1],[K0|K1],[Q2|K2],[K2d|Q2d]
            v_sb = pp.tile([128, NKT, 384], BF16, tag="vaug")  # per kt: 3x [v_h(64) | ones(64)]

            nc.gpsimd.memset(actbias_sb[:], ACT_BIAS)
            # warm the exp table-set early (one tiny activate) while DMAs run
            nc.scalar.activation(warm_sb[:], actbias_sb[:].to_broadcast((128, 8)), Exp)

            nc.sync.dma_start(wqk_sb[:], wqk.rearrange("(o p) m -> p o m", p=128))
            nc.sync.dma_start(bqk_sb[:], bqk.rearrange("(m p) -> p m", p=128))
            # xT arrives in (token-chunk, kc) granules so compute starts early
            for qq in range(NQC):
                for kc in range(KC):
                    nc.sync.dma_start(
                        xT_sb[:, kc, QC * qq : QC * qq + QC],
                        xT[128 * kc : 128 * kc + 128, QC * qq : QC * qq + QC],
                    )
            nc.sync.dma_start(wv_sb[:], wv.rearrange("(o p) m -> p o m", p=128))
            nc.sync.dma_start(wp_sb[:, 0, :], wp[0:128, :])
            nc.sync.dma_start(wp_sb[0:64, 1, :], wp[128:192, :])
            nc.gpsimd.memset(v_sb[:], 1.0)

            def qk_phase(qc):
                # Q^T / K^T projection for one 512-token slice, + head-2 swap copy
                for mt in range(3):
                    ps_t = ps.tile([128, 2 * QC], F32, tag="sa", name="qkps")[:, 0:QC]
                    for kc in range(KC):
                        nc.tensor.matmul(
                            ps_t[:],
                            wqk_sb[:, kc, 128 * mt : 128 * mt + 128],
                            xT_sb[:, kc, QC * qc : QC * qc + QC],
                            start=(kc == 0),
                            stop=(kc == KC - 1),
                        )
                    nc.vector.tensor_scalar(
                        qk_sb[:, mt, QC * qc : QC * qc + QC],
                        ps_t[:],
                        bqk_sb[:, mt : mt + 1],
                        None,
                        Alu.add,
                    )
                sl = slice(QC * qc, QC * qc + QC)
                nc.sync.dma_start(qk_sb[0:64, 3, sl], qk_sb[64:128, 2, sl])
                nc.sync.dma_start(qk_sb[64:128, 3, sl], qk_sb[0:64, 2, sl])

            def v_phase():
                for kt in range(NKT):
                    ps_t = ps.tile([128, 2 * QC], F32, tag="sa", name="vps")[:, 0:192]
                    for kc in range(KC):
                        nc.tensor.matmul(
                            ps_t[:],
                            xT_sb[:, kc, KT * kt : KT * kt + KT],
                            wv_sb[:, kc, :],
                            start=(kc == 0),
                            stop=(kc == KC - 1),
                        )
                    nc.vector.tensor_copy(
                        out=v_sb[:, kt, :].rearrange("p (h c) -> p h c", c=128)[:, :, 0:64],
                        in_=ps_t[:].rearrange("p (h c) -> p h c", c=64),
                    )

            # score matmul operands: heads 0/1 pair on partition halves; head 2
            # alternates halves by kt parity via the swapped copy in slot 3.
            def s_operands(h, kt):
                if h < 2:
                    po = 64 * h
                    return (1, po), (0, po)
                return ((3, 0) if kt % 2 == 0 else (2, 64)), ((2, 0) if kt % 2 == 0 else (3, 64))

            def s_mm(dst, h, kt, qc):
                (lm, lp), (rm, rp) = s_operands(h, kt)
                nc.tensor.matmul(
                    dst,
                    qk_sb[lp : lp + 64, lm, KT * kt : KT * kt + KT],
                    qk_sb[rp : rp + 64, rm, QC * qc : QC * qc + QC],
                    start=True,
                    stop=True,
                    tile_position=(lp, 0),
                )

            def attention(qc):
                o_a = op_.tile([128, QC], BF16, tag="oa", name="oa")
                o_b = op_.tile([64, QC], BF16, tag="ob", name="ob")
                for heads in ((0, 1), (2,)):
                    o_ps = {
                        h: ps_acc.tile([128, QC], F32, tag="acc", name="acc")
                        for h in heads
                    }
                    steps = (
                        [((heads[0], kt, 0), (heads[1], kt, QC)) for kt in range(NKT)]
                        if len(heads) == 2
                        else [((2, 2 * i, 0), (2, 2 * i + 1, QC)) for i in range(NKT // 2)]
                    )
                    for step in steps:
                        s2 = ps.tile([128, 2 * QC], F32, tag="sa", name="sa")
                        for h, kt, off in step:
                            s_mm(s2[:, off : off + QC], h, kt, qc)
                        pt = pta.tile([128, 2 * QC], BF16, tag="pta", name="pta")
                        nc.scalar.activation(pt[:], s2[:], Exp, bias=actbias_sb[:])
                        for h, kt, off in step:
                            nc.tensor.matmul(
                                o_ps[h][:],
                                v_sb[:, kt, 128 * h : 128 * h + 128],
                                pt[:, off : off + QC],
                                start=(kt == 0),
                                stop=(kt == NKT - 1),
                            )
                    # evacuate accumulators fast, normalize from the copy
                    for h in heads:
                        oc = sp.tile([128, QC], F32, tag="ocopy", name="ocopy")
                        nc.vector.tensor_copy(out=oc[:], in_=o_ps[h][:])
                        rec = sp.tile([64, QC], F32, tag="recip", name="recip")
                        nc.vector.reciprocal(out=rec[:], in_=oc[64:128, :])
                        dst = o_a[64 * h : 64 * h + 64, :] if h < 2 else o_b[:]
                        nc.vector.tensor_tensor(dst, oc[0:64, :], rec[:], Alu.mult)
                return o_a, o_b

            def proj(qc, o_a, o_b):
                for qt in range(QC // 128):
                    ys = yp.tile([128, DIM], F32, tag="y", name="y")
                    for nc2 in range(2):
                        nsl = slice(384 * nc2, 384 * nc2 + 384)
                        yps = ps.tile([128, 2 * QC], F32, tag="sa", name="yps")[:, 0:384]
                        nc.tensor.matmul(
                            yps[:], o_a[:, 128 * qt : 128 * qt + 128], wp_sb[:, 0, nsl],
                            start=True, stop=False,
                        )
                        nc.tensor.matmul(
                            yps[:], o_b[:, 128 * qt : 128 * qt + 128], wp_sb[0:64, 1, nsl],
                            start=False, stop=True,
                        )
                        nc.vector.tensor_copy(out=ys[:, nsl], in_=yps[:])
                    nc.sync.dma_start(
                        out[QC * qc + 128 * qt : QC * qc + 128 * qt + 128, :], ys[:]
                    )

            # ---- software-pipelined schedule ----
            # K^T and V cover all tokens, so the whole projection phase must
            # precede attention; only the output projection is delayed by one
            # chunk so its PSUM evict + matmuls overlap the next chunk.
            qk_phase(0)
            v_phase()
            for qc in range(1, NQC):
                qk_phase(qc)
            prev = None
            for qc in range(NQC):
                o_ab = attention(qc)
                if prev is not None:
                    proj(qc - 1, *prev)  # normalize of qc-1 ran on DVE meanwhile
                prev = o_ab
            proj(NQC - 1, *prev)
    return nc


_NC_CACHE = {}


def _get_nc():
    if "nc" not in _NC_CACHE:
        _NC_CACHE["nc"] = build_kernel()
    return _NC_CACHE["nc"]


def kernel(x, qkv_w, qkv_b, proj_w, proj_b):
    x = np.asarray(x, np.float32)
    qkv_w = np.asarray(qkv_w, np.float32)
    qkv_b = np.asarray(qkv_b, np.float32)
    proj_w = np.asarray(proj_w, np.float32)
    proj_b = np.asarray(proj_b, np.float32)

    wr = qkv_w.reshape(DIM, 3, H, Dh)
    br = qkv_b.reshape(3, H, Dh)
    scale = Dh ** -0.5

    in_maps = []
    for core in range(NCORES):
        b, g = divmod(core, 4)
        hs = slice(G * g, G * g + G)
        wq = wr[:, 0, hs, :].reshape(DIM, G * Dh) * scale  # fold softmax scale into Q
        wk = wr[:, 1, hs, :].reshape(DIM, G * Dh)
        wvm = wr[:, 2, hs, :].reshape(DIM, G * Dh)
        bq = br[0, hs].reshape(G * Dh) * scale
        bk = br[1, hs].reshape(G * Dh)
        # column order: mt0=[Q0|Q1], mt1=[K0|K1], mt2=[Q2|K2] (64 cols per head)
        wqk_c = np.concatenate(
            [wq[:, 0:128], wk[:, 0:128], wq[:, 128:192], wk[:, 128:192]], axis=1
        )
        bqk_c = np.concatenate([bq[0:128], bk[0:128], bq[128:192], bk[128:192]])
        in_maps.append(
            {
                "xT": np.ascontiguousarray(x[b].T).astype(bf16),
                "wqk": np.ascontiguousarray(wqk_c).astype(bf16),
                "bqk": np.ascontiguousarray(bqk_c),
                "wv": np.ascontiguousarray(wvm).astype(bf16),
                "wp": np.ascontiguousarray(proj_w[64 * G * g : 64 * G * (g + 1), :]).astype(bf16),
            }
        )

    nc = _get_nc()
    res = run_bass_kernel_spmd(nc, in_maps, core_ids=list(range(NCORES)))
    _NC_CACHE["last_result"] = res

    bias_row = (br[2].reshape(DIM).astype(np.float64) @ proj_w.astype(np.float64)
                + proj_b.astype(np.float64)).astype(np.float32)
    out = np.zeros((B, N, DIM), np.float32)
    for b in range(B):
        acc = np.zeros((N, DIM), np.float64)
        for g in range(4):
            acc += res.results[4 * b + g]["out"].astype(np.float64)
        out[b] = acc.astype(np.float32) + bias_row
    return out


# revision 19
# speedup vs baseline: 1.3023x; 1.0727x over previous
"""Trainium2 Bass kernel: multi-head attention (B=2, N=2048, DIM=768, H=12, Dh=64),
sharded (batch x head-group) across 8 NeuronCores. Self-contained.

Per-core shard (core = b*4 + g, g in 0..3, heads 3g..3g+2):
  - computes Q^T,K^T (features on partitions) and V (tokens on partitions) from x[b]^T
  - scores S^T[k,q] per head via row-tiled K=64 matmuls (2 concurrent per slot)
  - exp via ScalarE (table exp) + VectorE (custom cubic^4 approx), split per k-tile
  - O~^T/sums via augmented-V matmul (64 ones columns => sums broadcast on partitions 64:128)
  - normalize with approx reciprocal, project with proj_w rows, partial out [2048, 768] f32
Host: shards inputs, gathers partials: out[b] = sum_g partial + (b_v @ proj_w + proj_b).
"""

import sys

for _p in ("/opt/trn_rl_repo",):
    if _p not in sys.path:
        sys.path.append(_p)

import numpy as np
import ml_dtypes

import concourse.bass as bass
import concourse.mybir as mybir
import concourse.tile as tile
from concourse.bass_utils import run_bass_kernel_spmd

BF16 = mybir.dt.bfloat16
F32 = mybir.dt.float32
bf16 = ml_dtypes.bfloat16

B, N, DIM = 2, 2048, 768
H, Dh = 12, 64
G = 3  # heads per core
NCORES = 8
QC = 512  # query chunk (free dim of score matmuls)
NQC = N // QC
KT = 128  # key tile (partition dim of S^T)
NKT = N // KT

# exp split: which k-tiles go to the VectorE (custom poly) vs ScalarE (table exp).
# DVE k-tiles are singles; ACT k-tiles are grouped in pairs of 2 (one [128,1024] inst).
# NOTE: custom DVE ops fail to encode in this container's walrus ("ISA wrong
# length"), so all exp goes through ScalarE for now.
DVE_KTS = ()
ACT_PAIRS = tuple((2 * i, 2 * i + 1) for i in range(8))

# EXP4 constants: exp(x) ~ C0F^4 * ((1+A x)(1 + B x + CC x^2))^4 on |x| <= 2.75
EXP_A = 0.14770726095997042
EXP_B = 0.10315315610745052
EXP_CC = 0.017226206106509708
EXP_C0F = 0.9990441257079289
ACT_BIAS = -4.0 * float(np.log(EXP_C0F))  # ScalarE computes exp(x + bias) to match


# --------------------------------------------------------------------------
# workaround: this container's walrus accepts only ONE sync-wait per
# instruction ("Too many sync wait commands"). Split multi-wait sync_infos
# onto same-engine NoOps inserted right before the instruction.
def _patch_to_json():
    import orjson

    if getattr(bass.Bass, "_ant_json_patched", False):
        return
    orig = bass.Bass.to_json_bytes

    def to_json_bytes(self, *a, **kw):
        m = orjson.loads(orig(self, *a, **kw))

        def walk(o):
            if isinstance(o, dict):
                insts = o.get("instructions")
                if isinstance(insts, list) and insts and isinstance(insts[0], dict):
                    new = []
                    for inst in insts:
                        si = inst.get("sync_info")
                        waits = (si or {}).get("on_wait") or []
                        if len(waits) > 1:
                            for i, w in enumerate(waits[:-1]):
                                new.append(
                                    {
                                        "debug": inst.get("debug", 0),
                                        "engine": inst["engine"],
                                        "ins": [],
                                        "name": f"{inst['name']}-sw{i}",
                                        "opcode": "NoOp",
                                        "outs": [],
                                        "sync_info": {
                                            "on_update": [],
                                            "on_wait": [w],
                                        },
                                    }
                                )
                            si["on_wait"] = waits[-1:]
                        new.append(inst)
                    o["instructions"] = new
                for v in o.values():
                    walk(v)
            elif isinstance(o, list):
                for v in o:
                    walk(v)

        walk(m)
        return orjson.dumps(m)

    bass.Bass.to_json_bytes = to_json_bytes
    bass.Bass._ant_json_patched = True


# workaround: this container's walrus allows only 1 sync-wait on SP CTRL ops;
# Tile's kernel-tail drain piles every outstanding proc wait onto one Drain.
def _patch_tile_drain():
    from concourse.tile import TileContext, ScopedClock

    if getattr(TileContext, "_ant_drain_patched", False):
        return

    def _drain_and_barrier(self, tick_clock, wait_clock):
        nc = self.nc
        collector = nc.sync.nop(nofuse=True)
        wait_clock.add_sem_waits(
            collector.ins, ScopedClock({None: tick_clock.global_clock})
        )
        si = collector.ins.sync_info
        waits = list(si.on_wait) if si is not None else []
        if len(waits) > 1:
            si.on_wait = waits[:1]
            for w in waits[1:]:
                extra = nc.sync.nop(nofuse=True)
                extra.ins.sync_info = mybir.SyncInfo(on_wait=[w], on_update=[])
        nc.sync.drain()
        nc.all_engine_barrier()
        assert self.sems is not None
        popped = nc._tile_sem_poison_stack.pop()
        assert popped is self._sem_poison
        nc.clear_and_free_semaphores(list(self.sems.allocated().values()))
        nc.all_engine_barrier()

    TileContext._drain_and_barrier = _drain_and_barrier
    TileContext._ant_drain_patched = True


# --------------------------------------------------------------------------
# custom DVE ops: cubic ~ exp(x/4)/C0F (1 pass) and x -> x^4 (1 pass)
_EXP_OPS = {}


def _register_exp_ops():
    if _EXP_OPS:
        return _EXP_OPS
    from concourse import dve_ops
    from concourse.dve_ops import DveOp, OPS, _SUB_OPCODE_FOR_NAME
    from concourse.dve_spec import Spec, Src0, C0, C1, C2, One, sq, lower
    from concourse.dve_uop import DveOpSpec

    def make(name, spec):
        if name in _SUB_OPCODE_FOR_NAME:
            for op in OPS:
                if op.name == name:
                    return op
        row = max(_SUB_OPCODE_FOR_NAME.values()) + 1
        op = DveOp(name, spec, subdim=False, uops_sha={})
        OPS.append(op)
        _SUB_OPCODE_FOR_NAME[name] = row
        dve_ops.CUSTOM_DVE_SPECS[name] = spec
        for ver in ("v3", "v4"):
            uops = lower(spec, ver=ver)
            op.uops_sha[ver] = DveOpSpec(
                name=name, opcode=row, uops=uops, rd1_en=False
            ).sha(ver)
        return op

    cubic = make(
        "EXPC_ANT",
        Spec(
            body=(Src0 * C0 + One) * ((sq(Src0) * C2 + Src0 * C1) + One),
            reference=lambda in0, in1, s0, s1, imm2: (in0 * s0 + 1.0)
            * ((in0 * in0) * imm2 + in0 * s1 + 1.0),
        ),
    )
    pow4 = make(
        "POW4_ANT",
        Spec(
            body=sq(sq(Src0)),
            reference=lambda in0, in1, s0, s1, imm2: (in0 * in0) * (in0 * in0),
        ),
    )
    _EXP_OPS["cubic"] = cubic
    _EXP_OPS["pow4"] = pow4
    return _EXP_OPS


# --------------------------------------------------------------------------
def build_kernel():
    _patch_to_json()
    _patch_tile_drain()
    Exp = mybir.ActivationFunctionType.Exp
    Alu = mybir.AluOpType

    nc = bass.Bass(trn_type="TRN2")
    xT = nc.dram_tensor("xT", [DIM, N], BF16, kind="ExternalInput")
    wqk = nc.dram_tensor("wqk", [DIM, 384], BF16, kind="ExternalInput")
    bqk = nc.dram_tensor("bqk", [384], F32, kind="ExternalInput")
    wv = nc.dram_tensor("wv", [DIM, 192], BF16, kind="ExternalInput")
    wp = nc.dram_tensor("wp", [192, DIM], BF16, kind="ExternalInput")
    out = nc.dram_tensor("out", [N, DIM], F32, kind="ExternalOutput")

    KC = DIM // 128  # 6 contraction chunks

    with tile.TileContext(nc) as tc:
        with (
            tc.tile_pool(name="persist", bufs=1) as pp,
            tc.tile_pool(name="pt_act", bufs=4) as pta,
            tc.tile_pool(name="scratch", bufs=4) as sp,
            tc.tile_pool(name="osb", bufs=3) as op_,
            tc.tile_pool(name="ysb", bufs=3) as yp,
            tc.tile_pool(name="ps", bufs=3, space="PSUM") as ps,
            tc.tile_pool(name="ps_acc", bufs=2, space="PSUM") as ps_acc,
        ):
            # ---- persistent SBUF ----
            xT_sb = pp.tile([128, KC, N], BF16, tag="xT")
            wqk_sb = pp.tile([128, KC, 384], BF16, tag="wqk")
            wv_sb = pp.tile([128, KC, 192], BF16, tag="wv")
            wp_sb = pp.tile([128, 2, DIM], BF16, tag="wp")
            bqk_sb = pp.tile([128, 3], F32, tag="bqk")
            actbias_sb = pp.tile([128, 1], F32, tag="actbias")
            warm_sb = pp.tile([128, 8], BF16, tag="warm")
            qk_sb = pp.tile([128, 4, N], BF16, tag="qkT")  # mt: [Q0|Q1],[K0|K1],[Q2|K2],[K2d|Q2d]
            v_sb = pp.tile([128, NKT, 384], BF16, tag="vaug")  # per kt: 3x [v_h(64) | ones(64)]

            nc.gpsimd.memset(actbias_sb[:], ACT_BIAS)
            # warm the exp table-set early (one tiny activate) while DMAs run
            nc.scalar.activation(warm_sb[:], actbias_sb[:].to_broadcast((128, 8)), Exp)

            nc.sync.dma_start(wqk_sb[:], wqk.rearrange("(o p) m -> p o m", p=128))
            nc.sync.dma_start(bqk_sb[:], bqk.rearrange("(m p) -> p m", p=128))
            # xT arrives in (token-chunk, kc) granules so compute starts early
            for qq in range(NQC):
                for kc in range(KC):
                    eng = nc.sync if kc % 2 == 0 else nc.gpsimd
                    eng.dma_start(
                        xT_sb[:, kc, QC * qq : QC * qq + QC],
                        xT[128 * kc : 128 * kc + 128, QC * qq : QC * qq + QC],
                    )
            nc.sync.dma_start(wv_sb[:], wv.rearrange("(o p) m -> p o m", p=128))
            nc.sync.dma_start(wp_sb[:, 0, :], wp[0:128, :])
            nc.sync.dma_start(wp_sb[0:64, 1, :], wp[128:192, :])
            nc.gpsimd.memset(v_sb[:], 1.0)

            def qk_phase(qc):
                # Q^T / K^T projection for one 512-token slice, + head-2 swap copy
                for mt in range(3):
                    ps_t = ps.tile([128, 2 * QC], F32, tag="sa", name="qkps")[:, 0:QC]
                    for kc in range(KC):
                        nc.tensor.matmul(
                            ps_t[:],
                            wqk_sb[:, kc, 128 * mt : 128 * mt + 128],
                            xT_sb[:, kc, QC * qc : QC * qc + QC],
                            start=(kc == 0),
                            stop=(kc == KC - 1),
                        )
                    nc.vector.tensor_scalar(
                        qk_sb[:, mt, QC * qc : QC * qc + QC],
                        ps_t[:],
                        bqk_sb[:, mt : mt + 1],
                        None,
                        Alu.add,
                    )
                sl = slice(QC * qc, QC * qc + QC)
                nc.sync.dma_start(qk_sb[0:64, 3, sl], qk_sb[64:128, 2, sl])
                nc.sync.dma_start(qk_sb[64:128, 3, sl], qk_sb[0:64, 2, sl])

            def v_tile(kt):
                    ps_t = ps.tile([128, 2 * QC], F32, tag="sa", name="vps")[:, 0:192]
                    for kc in range(KC):
                        nc.tensor.matmul(
                            ps_t[:],
                            xT_sb[:, kc, KT * kt : KT * kt + KT],
                            wv_sb[:, kc, :],
                            start=(kc == 0),
                            stop=(kc == KC - 1),
                        )
                    nc.vector.tensor_copy(
                        out=v_sb[:, kt, :].rearrange("p (h c) -> p h c", c=128)[:, :, 0:64],
                        in_=ps_t[:].rearrange("p (h c) -> p h c", c=64),
                    )

            # score matmul operands: heads 0/1 pair on partition halves; head 2
            # alternates halves by kt parity via the swapped copy in slot 3.
            def s_operands(h, kt):
                if h < 2:
                    po = 64 * h
                    return (1, po), (0, po)
                return ((3, 0) if kt % 2 == 0 else (2, 64)), ((2, 0) if kt % 2 == 0 else (3, 64))

            def s_mm(dst, h, kt, qc):
                (lm, lp), (rm, rp) = s_operands(h, kt)
                nc.tensor.matmul(
                    dst,
                    qk_sb[lp : lp + 64, lm, KT * kt : KT * kt + KT],
                    qk_sb[rp : rp + 64, rm, QC * qc : QC * qc + QC],
                    start=True,
                    stop=True,
                    tile_position=(lp, 0),
                )

            def attention(qc):
                o_a = op_.tile([128, QC], BF16, tag="oa", name="oa")
                o_b = op_.tile([64, QC], BF16, tag="ob", name="ob")
                ocs = []
                for heads in ((0, 1), (2,)):
                    o_ps = {
                        h: ps_acc.tile([128, QC], F32, tag="acc", name="acc")
                        for h in heads
                    }
                    steps = (
                        [((heads[0], kt, 0), (heads[1], kt, QC)) for kt in range(NKT)]
                        if len(heads) == 2
                        else [((2, 2 * i, 0), (2, 2 * i + 1, QC)) for i in range(NKT // 2)]
                    )
                    for step in steps:
                        s2 = ps.tile([128, 2 * QC], F32, tag="sa", name="sa")
                        for h, kt, off in step:
                            s_mm(s2[:, off : off + QC], h, kt, qc)
                        pt = pta.tile([128, 2 * QC], BF16, tag="pta", name="pta")
                        nc.scalar.activation(pt[:], s2[:], Exp, bias=actbias_sb[:])
                        for h, kt, off in step:
                            nc.tensor.matmul(
                                o_ps[h][:],
                                v_sb[:, kt, 128 * h : 128 * h + 128],
                                pt[:, off : off + QC],
                                start=(kt == 0),
                                stop=(kt == NKT - 1),
                            )
                    # evacuate accumulators immediately (frees the PSUM bank)
                    for h in heads:
                        oc = sp.tile([128, QC], F32, tag="ocopy", name="ocopy")
                        nc.vector.tensor_copy(out=oc[:], in_=o_ps[h][:])
                        ocs.append((h, oc))
                return o_a, o_b, ocs

            MAGIC = 0x7EF311C3

            def normalize(o_a, o_b, ocs, last=False):
                # rec = -(approx 1/sums): seed via int bit trick + 1 Newton step.
                # The sign is fixed up on the host (partials are negated).
                for h, oc in ocs:
                    seedt = sp.tile([128, QC], F32, tag="seed", name="seed")
                    seed = seedt[64:128, :]
                    nc.vector.tensor_scalar(
                        seed.bitcast(mybir.dt.int32),
                        oc[64:128, :].bitcast(mybir.dt.int32),
                        MAGIC, -1, Alu.subtract, Alu.mult,
                    )
                    ut = sp.tile([128, QC], F32, tag="nru", name="nru")
                    u = ut[64:128, :]
                    nc.vector.tensor_tensor(u, oc[64:128, :], seed, Alu.mult)
                    rect = sp.tile([128, QC], F32, tag="recip", name="recip")
                    rec = rect[0:64, :]
                    # rec = (u - 2) * seed = -(1/sums approx); out base differs from
                    # in base, which is allowed for single-tensor-input ops only,
                    # so route through scalar_tensor_tensor on matching halves
                    # then multiply from the low half.
                    nc.vector.scalar_tensor_tensor(
                        rect[64:128, :], u, 2.0, seed, Alu.subtract, Alu.mult
                    )
                    nc.vector.tensor_copy(out=rec, in_=rect[64:128, :])
                    dst = o_a[64 * h : 64 * h + 64, :] if h < 2 else o_b[:]
                    nc.vector.tensor_tensor(dst, oc[0:64, :], rec, Alu.mult)

            def proj(qc, o_a, o_b, act_evict=False):
                for qt in range(QC // 128):
                    ys = yp.tile([128, DIM], F32, tag="y", name="y")
                    for nc2 in range(2):
                        nsl = slice(384 * nc2, 384 * nc2 + 384)
                        yps = ps.tile([128, 2 * QC], F32, tag="sa", name="yps")[:, 0:384]
                        nc.tensor.matmul(
                            yps[:], o_a[:, 128 * qt : 128 * qt + 128], wp_sb[:, 0, nsl],
                            start=True, stop=False,
                        )
                        nc.tensor.matmul(
                            yps[:], o_b[:, 128 * qt : 128 * qt + 128], wp_sb[0:64, 1, nsl],
                            start=False, stop=True,
                        )
                        if act_evict and nc2 == 1:
                            nc.scalar.copy(ys[:, nsl], yps[:])
                        else:
                            nc.vector.tensor_copy(out=ys[:, nsl], in_=yps[:])
                    nc.gpsimd.dma_start(
                        out[QC * qc + 128 * qt : QC * qc + 128 * qt + 128, :], ys[:]
                    )

            # ---- software-pipelined schedule ----
            # Warm the PE clock (HAM) with throwaway matmuls on zeroed SBUF
            # while the input DMAs stream in.
            warm_in = pp.tile([128, 256], BF16, tag="warmmm")
            nc.gpsimd.memset(warm_in[:], 0.0)
            wps = ps.tile([128, 2 * QC], F32, tag="sa", name="warmps")[:, 0:256]
            for i in range(48):
                nc.tensor.matmul(wps[:], warm_in[:, 0:128], warm_in[:],
                                 start=(i == 0), stop=(i == 47))

            # K^T and V cover all tokens, so the whole projection phase must
            # precede attention (interleaved to match the DMA arrival order);
            # the output projection is delayed by one chunk, and each chunk's
            # normalize is emitted after it so the DVE drains evictions first.
            for qq in range(NQC):
                qk_phase(qq)
                for kt in range(4 * qq, 4 * qq + 4):
                    v_tile(kt)
            prev = None
            for qc in range(NQC):
                o_ab = attention(qc)
                if prev is not None:
                    proj(qc - 1, prev[0], prev[1])
                    normalize(*o_ab)
                else:
                    normalize(*o_ab)
                prev = o_ab
            proj(NQC - 1, prev[0], prev[1], act_evict=True)
    return nc


_NC_CACHE = {}


def _get_nc():
    if "nc" not in _NC_CACHE:
        _NC_CACHE["nc"] = build_kernel()
    return _NC_CACHE["nc"]


def kernel(x, qkv_w, qkv_b, proj_w, proj_b):
    x = np.asarray(x, np.float32)
    qkv_w = np.asarray(qkv_w, np.float32)
    qkv_b = np.asarray(qkv_b, np.float32)
    proj_w = np.asarray(proj_w, np.float32)
    proj_b = np.asarray(proj_b, np.float32)

    wr = qkv_w.reshape(DIM, 3, H, Dh)
    br = qkv_b.reshape(3, H, Dh)
    scale = Dh ** -0.5

    in_maps = []
    for core in range(NCORES):
        b, g = divmod(core, 4)
        hs = slice(G * g, G * g + G)
        wq = wr[:, 0, hs, :].reshape(DIM, G * Dh) * scale  # fold softmax scale into Q
        wk = wr[:, 1, hs, :].reshape(DIM, G * Dh)
        wvm = wr[:, 2, hs, :].reshape(DIM, G * Dh)
        bq = br[0, hs].reshape(G * Dh) * scale
        bk = br[1, hs].reshape(G * Dh)
        # column order: mt0=[Q0|Q1], mt1=[K0|K1], mt2=[Q2|K2] (64 cols per head)
        wqk_c = np.concatenate(
            [wq[:, 0:128], wk[:, 0:128], wq[:, 128:192], wk[:, 128:192]], axis=1
        )
        bqk_c = np.concatenate([bq[0:128], bk[0:128], bq[128:192], bk[128:192]])
        in_maps.append(
            {
                "xT": np.ascontiguousarray(x[b].T).astype(bf16),
                "wqk": np.ascontiguousarray(wqk_c).astype(bf16),
                "bqk": np.ascontiguousarray(bqk_c),
                "wv": np.ascontiguousarray(wvm).astype(bf16),
                "wp": np.ascontiguousarray(proj_w[64 * G * g : 64 * G * (g + 1), :]).astype(bf16),
            }
        )

    nc = _get_nc()
    res = run_bass_kernel_spmd(nc, in_maps, core_ids=list(range(NCORES)))
    _NC_CACHE["last_result"] = res

    bias_row = (br[2].reshape(DIM).astype(np.float64) @ proj_w.astype(np.float64)
                + proj_b.astype(np.float64)).astype(np.float32)
    out = np.zeros((B, N, DIM), np.float32)
    for b in range(B):
        acc = np.zeros((N, DIM), np.float64)
        for g in range(4):
            acc += res.results[4 * b + g]["out"].astype(np.float64)
        out[b] = (-acc).astype(np.float32) + bias_row
    return out


# revision 20
# speedup vs baseline: 1.3496x; 1.0363x over previous
"""Trainium2 Bass kernel: multi-head attention (B=2, N=2048, DIM=768, H=12, Dh=64),
sharded (batch x head-group) across 8 NeuronCores. Self-contained.

Per-core shard (core = b*4 + g, g in 0..3, heads 3g..3g+2):
  - computes Q^T,K^T (features on partitions) and V (tokens on partitions) from x[b]^T
  - scores S^T[k,q] per head via row-tiled K=64 matmuls (2 concurrent per slot)
  - exp via ScalarE (table exp) + VectorE (custom cubic^4 approx), split per k-tile
  - O~^T/sums via augmented-V matmul (64 ones columns => sums broadcast on partitions 64:128)
  - normalize with approx reciprocal, project with proj_w rows, partial out [2048, 768] f32
Host: shards inputs, gathers partials: out[b] = sum_g partial + (b_v @ proj_w + proj_b).
"""

import sys

for _p in ("/opt/trn_rl_repo",):
    if _p not in sys.path:
        sys.path.append(_p)

import numpy as np
import ml_dtypes

import concourse.bass as bass
import concourse.mybir as mybir
import concourse.tile as tile
from concourse.bass_utils import run_bass_kernel_spmd

BF16 = mybir.dt.bfloat16
F32 = mybir.dt.float32
bf16 = ml_dtypes.bfloat16

B, N, DIM = 2, 2048, 768
H, Dh = 12, 64
G = 3  # heads per core
NCORES = 8
QC = 512  # query chunk (free dim of score matmuls)
NQC = N // QC
KT = 128  # key tile (partition dim of S^T)
NKT = N // KT

# exp split: which k-tiles go to the VectorE (custom poly) vs ScalarE (table exp).
# DVE k-tiles are singles; ACT k-tiles are grouped in pairs of 2 (one [128,1024] inst).
# NOTE: custom DVE ops fail to encode in this container's walrus ("ISA wrong
# length"), so all exp goes through ScalarE for now.
DVE_KTS = ()
ACT_PAIRS = tuple((2 * i, 2 * i + 1) for i in range(8))

# EXP4 constants: exp(x) ~ C0F^4 * ((1+A x)(1 + B x + CC x^2))^4 on |x| <= 2.75
EXP_A = 0.14770726095997042
EXP_B = 0.10315315610745052
EXP_CC = 0.017226206106509708
EXP_C0F = 0.9990441257079289
ACT_BIAS = -4.0 * float(np.log(EXP_C0F))  # ScalarE computes exp(x + bias) to match


# --------------------------------------------------------------------------
# workaround: this container's walrus accepts only ONE sync-wait per
# instruction ("Too many sync wait commands"). Split multi-wait sync_infos
# onto same-engine NoOps inserted right before the instruction.
def _patch_to_json():
    import orjson

    if getattr(bass.Bass, "_ant_json_patched", False):
        return
    orig = bass.Bass.to_json_bytes

    def to_json_bytes(self, *a, **kw):
        m = orjson.loads(orig(self, *a, **kw))

        def walk(o):
            if isinstance(o, dict):
                insts = o.get("instructions")
                if isinstance(insts, list) and insts and isinstance(insts[0], dict):
                    new = []
                    for inst in insts:
                        si = inst.get("sync_info")
                        waits = (si or {}).get("on_wait") or []
                        if len(waits) > 1:
                            for i, w in enumerate(waits[:-1]):
                                new.append(
                                    {
                                        "debug": inst.get("debug", 0),
                                        "engine": inst["engine"],
                                        "ins": [],
                                        "name": f"{inst['name']}-sw{i}",
                                        "opcode": "NoOp",
                                        "outs": [],
                                        "sync_info": {
                                            "on_update": [],
                                            "on_wait": [w],
                                        },
                                    }
                                )
                            si["on_wait"] = waits[-1:]
                        new.append(inst)
                    o["instructions"] = new
                for v in o.values():
                    walk(v)
            elif isinstance(o, list):
                for v in o:
                    walk(v)

        walk(m)
        return orjson.dumps(m)

    bass.Bass.to_json_bytes = to_json_bytes
    bass.Bass._ant_json_patched = True


# workaround: this container's walrus allows only 1 sync-wait on SP CTRL ops;
# Tile's kernel-tail drain piles every outstanding proc wait onto one Drain.
def _patch_tile_drain():
    from concourse.tile import TileContext, ScopedClock

    if getattr(TileContext, "_ant_drain_patched", False):
        return

    def _drain_and_barrier(self, tick_clock, wait_clock):
        nc = self.nc
        collector = nc.sync.nop(nofuse=True)
        wait_clock.add_sem_waits(
            collector.ins, ScopedClock({None: tick_clock.global_clock})
        )
        si = collector.ins.sync_info
        waits = list(si.on_wait) if si is not None else []
        if len(waits) > 1:
            si.on_wait = waits[:1]
            for w in waits[1:]:
                extra = nc.sync.nop(nofuse=True)
                extra.ins.sync_info = mybir.SyncInfo(on_wait=[w], on_update=[])
        nc.sync.drain()
        nc.all_engine_barrier()
        assert self.sems is not None
        popped = nc._tile_sem_poison_stack.pop()
        assert popped is self._sem_poison
        nc.clear_and_free_semaphores(list(self.sems.allocated().values()))
        nc.all_engine_barrier()

    TileContext._drain_and_barrier = _drain_and_barrier
    TileContext._ant_drain_patched = True


# --------------------------------------------------------------------------
# custom DVE ops: cubic ~ exp(x/4)/C0F (1 pass) and x -> x^4 (1 pass)
_EXP_OPS = {}


def _register_exp_ops():
    if _EXP_OPS:
        return _EXP_OPS
    from concourse import dve_ops
    from concourse.dve_ops import DveOp, OPS, _SUB_OPCODE_FOR_NAME
    from concourse.dve_spec import Spec, Src0, C0, C1, C2, One, sq, lower
    from concourse.dve_uop import DveOpSpec

    def make(name, spec):
        if name in _SUB_OPCODE_FOR_NAME:
            for op in OPS:
                if op.name == name:
                    return op
        row = max(_SUB_OPCODE_FOR_NAME.values()) + 1
        op = DveOp(name, spec, subdim=False, uops_sha={})
        OPS.append(op)
        _SUB_OPCODE_FOR_NAME[name] = row
        dve_ops.CUSTOM_DVE_SPECS[name] = spec
        for ver in ("v3", "v4"):
            uops = lower(spec, ver=ver)
            op.uops_sha[ver] = DveOpSpec(
                name=name, opcode=row, uops=uops, rd1_en=False
            ).sha(ver)
        return op

    cubic = make(
        "EXPC_ANT",
        Spec(
            body=(Src0 * C0 + One) * ((sq(Src0) * C2 + Src0 * C1) + One),
            reference=lambda in0, in1, s0, s1, imm2: (in0 * s0 + 1.0)
            * ((in0 * in0) * imm2 + in0 * s1 + 1.0),
        ),
    )
    pow4 = make(
        "POW4_ANT",
        Spec(
            body=sq(sq(Src0)),
            reference=lambda in0, in1, s0, s1, imm2: (in0 * in0) * (in0 * in0),
        ),
    )
    _EXP_OPS["cubic"] = cubic
    _EXP_OPS["pow4"] = pow4
    return _EXP_OPS


# --------------------------------------------------------------------------
def build_kernel():
    _patch_to_json()
    _patch_tile_drain()
    Exp = mybir.ActivationFunctionType.Exp
    Alu = mybir.AluOpType

    nc = bass.Bass(trn_type="TRN2")
    xT = nc.dram_tensor("xT", [DIM, N], BF16, kind="ExternalInput")
    wqk = nc.dram_tensor("wqk", [DIM, 384], BF16, kind="ExternalInput")
    bqk = nc.dram_tensor("bqk", [384], F32, kind="ExternalInput")
    wv = nc.dram_tensor("wv", [DIM, 192], BF16, kind="ExternalInput")
    wp = nc.dram_tensor("wp", [192, DIM], BF16, kind="ExternalInput")
    out = nc.dram_tensor("out", [N, DIM], F32, kind="ExternalOutput")

    KC = DIM // 128  # 6 contraction chunks

    with tile.TileContext(nc) as tc:
        with (
            tc.tile_pool(name="persist", bufs=1) as pp,
            tc.tile_pool(name="pt_act", bufs=4) as pta,
            tc.tile_pool(name="scratch", bufs=4) as sp,
            tc.tile_pool(name="osb", bufs=3) as op_,
            tc.tile_pool(name="ysb", bufs=3) as yp,
            tc.tile_pool(name="ps", bufs=3, space="PSUM") as ps,
            tc.tile_pool(name="ps_acc", bufs=2, space="PSUM") as ps_acc,
        ):
            # ---- persistent SBUF ----
            xT_sb = pp.tile([128, KC, N], BF16, tag="xT")
            wqk_sb = pp.tile([128, KC, 384], BF16, tag="wqk")
            wv_sb = pp.tile([128, KC, 192], BF16, tag="wv")
            wp_sb = pp.tile([128, 2, DIM], BF16, tag="wp")
            bqk_sb = pp.tile([128, 3], F32, tag="bqk")
            actbias_sb = pp.tile([128, 1], F32, tag="actbias")
            warm_sb = pp.tile([128, 8], BF16, tag="warm")
            qk_sb = pp.tile([128, 4, N], BF16, tag="qkT")  # mt: [Q0|Q1],[K0|K1],[Q2|K2],[K2d|Q2d]
            v_sb = pp.tile([128, NKT, 384], BF16, tag="vaug")  # per kt: 3x [v_h(64) | ones(64)]

            # PE clock (HAM) warmup on zeroed SBUF + early exp-table load,
            # all before the heavyweight DMAs and memsets are queued.
            warm_in = pp.tile([128, 256], BF16, tag="warmmm")
            nc.gpsimd.memset(warm_in[:], 0.0)
            nc.gpsimd.memset(actbias_sb[:], ACT_BIAS)
            wps = ps.tile([128, 2 * QC], F32, tag="sa", name="warmps")[:, 0:256]
            for i in range(48):
                nc.tensor.matmul(wps[:], warm_in[:, 0:128], warm_in[:],
                                 start=(i == 0), stop=(i == 47))
            nc.scalar.activation(warm_sb[:], actbias_sb[:].to_broadcast((128, 8)), Exp)

            nc.sync.dma_start(wqk_sb[:], wqk.rearrange("(o p) m -> p o m", p=128))
            nc.sync.dma_start(bqk_sb[:], bqk.rearrange("(m p) -> p m", p=128))
            # xT arrives in (token-chunk, kc) granules so compute starts early
            for qq in range(NQC):
                for kc in range(KC):
                    eng = nc.sync if kc % 2 == 0 else nc.gpsimd
                    eng.dma_start(
                        xT_sb[:, kc, QC * qq : QC * qq + QC],
                        xT[128 * kc : 128 * kc + 128, QC * qq : QC * qq + QC],
                    )
            nc.sync.dma_start(wv_sb[:], wv.rearrange("(o p) m -> p o m", p=128))
            nc.sync.dma_start(wp_sb[:, 0, :], wp[0:128, :])
            nc.sync.dma_start(wp_sb[0:64, 1, :], wp[128:192, :])
            nc.vector.memset(v_sb[:], 1.0)

            def qk_phase(qc):
                # Q^T / K^T projection for one 512-token slice, + head-2 swap copy
                for mt in range(3):
                    ps_t = ps.tile([128, 2 * QC], F32, tag="sa", name="qkps")[:, 0:QC]
                    for kc in range(KC):
                        nc.tensor.matmul(
                            ps_t[:],
                            wqk_sb[:, kc, 128 * mt : 128 * mt + 128],
                            xT_sb[:, kc, QC * qc : QC * qc + QC],
                            start=(kc == 0),
                            stop=(kc == KC - 1),
                        )
                    nc.vector.tensor_scalar(
                        qk_sb[:, mt, QC * qc : QC * qc + QC],
                        ps_t[:],
                        bqk_sb[:, mt : mt + 1],
                        None,
                        Alu.add,
                    )
                sl = slice(QC * qc, QC * qc + QC)
                nc.sync.dma_start(qk_sb[0:64, 3, sl], qk_sb[64:128, 2, sl])
                nc.sync.dma_start(qk_sb[64:128, 3, sl], qk_sb[0:64, 2, sl])

            def v_tile(kt):
                    ps_t = ps.tile([128, 2 * QC], F32, tag="sa", name="vps")[:, 0:192]
                    for kc in range(KC):
                        nc.tensor.matmul(
                            ps_t[:],
                            xT_sb[:, kc, KT * kt : KT * kt + KT],
                            wv_sb[:, kc, :],
                            start=(kc == 0),
                            stop=(kc == KC - 1),
                        )
                    nc.vector.tensor_copy(
                        out=v_sb[:, kt, :].rearrange("p (h c) -> p h c", c=128)[:, :, 0:64],
                        in_=ps_t[:].rearrange("p (h c) -> p h c", c=64),
                    )

            # score matmul operands: heads 0/1 pair on partition halves; head 2
            # alternates halves by kt parity via the swapped copy in slot 3.
            def s_operands(h, kt):
                if h < 2:
                    po = 64 * h
                    return (1, po), (0, po)
                return ((3, 0) if kt % 2 == 0 else (2, 64)), ((2, 0) if kt % 2 == 0 else (3, 64))

            def s_mm(dst, h, kt, qc):
                (lm, lp), (rm, rp) = s_operands(h, kt)
                nc.tensor.matmul(
                    dst,
                    qk_sb[lp : lp + 64, lm, KT * kt : KT * kt + KT],
                    qk_sb[rp : rp + 64, rm, QC * qc : QC * qc + QC],
                    start=True,
                    stop=True,
                    tile_position=(lp, 0),
                )

            def attention(qc):
                o_a = op_.tile([128, QC], BF16, tag="oa", name="oa")
                o_b = op_.tile([64, QC], BF16, tag="ob", name="ob")
                ocs = []
                for heads in ((0, 1), (2,)):
                    o_ps = {
                        h: ps_acc.tile([128, QC], F32, tag="acc", name="acc")
                        for h in heads
                    }
                    steps = (
                        [((heads[0], kt, 0), (heads[1], kt, QC)) for kt in range(NKT)]
                        if len(heads) == 2
                        else [((2, 2 * i, 0), (2, 2 * i + 1, QC)) for i in range(NKT // 2)]
                    )
                    for step in steps:
                        s2 = ps.tile([128, 2 * QC], F32, tag="sa", name="sa")
                        for h, kt, off in step:
                            s_mm(s2[:, off : off + QC], h, kt, qc)
                        pt = pta.tile([128, 2 * QC], BF16, tag="pta", name="pta")
                        nc.scalar.activation(pt[:], s2[:], Exp, bias=actbias_sb[:])
                        for h, kt, off in step:
                            nc.tensor.matmul(
                                o_ps[h][:],
                                v_sb[:, kt, 128 * h : 128 * h + 128],
                                pt[:, off : off + QC],
                                start=(kt == 0),
                                stop=(kt == NKT - 1),
                            )
                    # evacuate accumulators immediately (frees the PSUM bank)
                    for h in heads:
                        oc = sp.tile([128, QC], F32, tag="ocopy", name="ocopy")
                        nc.vector.tensor_copy(out=oc[:], in_=o_ps[h][:])
                        ocs.append((h, oc))
                return o_a, o_b, ocs

            MAGIC = 0x7EF311C3

            def normalize(o_a, o_b, ocs, last=False):
                # rec = -(approx 1/sums): seed via int bit trick + 1 Newton step.
                # The sign is fixed up on the host (partials are negated).
                for h, oc in ocs:
                    seedt = sp.tile([128, QC], F32, tag="seed", name="seed")
                    seed = seedt[64:128, :]
                    nc.vector.tensor_scalar(
                        seed.bitcast(mybir.dt.int32),
                        oc[64:128, :].bitcast(mybir.dt.int32),
                        MAGIC, -1, Alu.subtract, Alu.mult,
                    )
                    ut = sp.tile([128, QC], F32, tag="nru", name="nru")
                    u = ut[64:128, :]
                    nc.vector.tensor_tensor(u, oc[64:128, :], seed, Alu.mult)
                    rect = sp.tile([128, QC], F32, tag="recip", name="recip")
                    rec = rect[0:64, :]
                    # rec = (u - 2) * seed = -(1/sums approx); out base differs from
                    # in base, which is allowed for single-tensor-input ops only,
                    # so route through scalar_tensor_tensor on matching halves
                    # then multiply from the low half.
                    nc.vector.scalar_tensor_tensor(
                        rect[64:128, :], u, 2.0, seed, Alu.subtract, Alu.mult
                    )
                    nc.vector.tensor_copy(out=rec, in_=rect[64:128, :])
                    dst = o_a[64 * h : 64 * h + 64, :] if h < 2 else o_b[:]
                    nc.vector.tensor_tensor(dst, oc[0:64, :], rec, Alu.mult)

            def proj(qc, o_a, o_b, act_evict=False):
                for qt in range(QC // 128):
                    ys = yp.tile([128, DIM], F32, tag="y", name="y")
                    for nc2 in range(2):
                        nsl = slice(384 * nc2, 384 * nc2 + 384)
                        yps = ps.tile([128, 2 * QC], F32, tag="sa", name="yps")[:, 0:384]
                        nc.tensor.matmul(
                            yps[:], o_a[:, 128 * qt : 128 * qt + 128], wp_sb[:, 0, nsl],
                            start=True, stop=False,
                        )
                        nc.tensor.matmul(
                            yps[:], o_b[:, 128 * qt : 128 * qt + 128], wp_sb[0:64, 1, nsl],
                            start=False, stop=True,
                        )
                        if act_evict and nc2 == 1:
                            nc.scalar.copy(ys[:, nsl], yps[:])
                        else:
                            nc.vector.tensor_copy(out=ys[:, nsl], in_=yps[:])
                    nc.gpsimd.dma_start(
                        out[QC * qc + 128 * qt : QC * qc + 128 * qt + 128, :], ys[:]
                    )

            # ---- software-pipelined schedule ----
            # K^T and V cover all tokens, so the whole projection phase must
            # precede attention (interleaved to match the DMA arrival order);
            # the output projection is delayed by one chunk, and each chunk's
            # normalize is emitted after it so the DVE drains evictions first.
            for qq in range(NQC):
                qk_phase(qq)
                for kt in range(4 * qq, 4 * qq + 4):
                    v_tile(kt)
            prev = None
            for qc in range(NQC):
                o_ab = attention(qc)
                if prev is not None:
                    proj(qc - 1, prev[0], prev[1])
                    normalize(*o_ab)
                else:
                    normalize(*o_ab)
                prev = o_ab
            proj(NQC - 1, prev[0], prev[1], act_evict=True)
    return nc


_NC_CACHE = {}


def _get_nc():
    if "nc" not in _NC_CACHE:
        _NC_CACHE["nc"] = build_kernel()
    return _NC_CACHE["nc"]


def kernel(x, qkv_w, qkv_b, proj_w, proj_b):
    x = np.asarray(x, np.float32)
    qkv_w = np.asarray(qkv_w, np.float32)
    qkv_b = np.asarray(qkv_b, np.float32)
    proj_w = np.asarray(proj_w, np.float32)
    proj_b = np.asarray(proj_b, np.float32)

    wr = qkv_w.reshape(DIM, 3, H, Dh)
    br = qkv_b.reshape(3, H, Dh)
    scale = Dh ** -0.5

    in_maps = []
    for core in range(NCORES):
        b, g = divmod(core, 4)
        hs = slice(G * g, G * g + G)
        wq = wr[:, 0, hs, :].reshape(DIM, G * Dh) * scale  # fold softmax scale into Q
        wk = wr[:, 1, hs, :].reshape(DIM, G * Dh)
        wvm = wr[:, 2, hs, :].reshape(DIM, G * Dh)
        bq = br[0, hs].reshape(G * Dh) * scale
        bk = br[1, hs].reshape(G * Dh)
        # column order: mt0=[Q0|Q1], mt1=[K0|K1], mt2=[Q2|K2] (64 cols per head)
        wqk_c = np.concatenate(
            [wq[:, 0:128], wk[:, 0:128], wq[:, 128:192], wk[:, 128:192]], axis=1
        )
        bqk_c = np.concatenate([bq[0:128], bk[0:128], bq[128:192], bk[128:192]])
        in_maps.append(
            {
                "xT": np.ascontiguousarray(x[b].T).astype(bf16),
                "wqk": np.ascontiguousarray(wqk_c).astype(bf16),
                "bqk": np.ascontiguousarray(bqk_c),
                "wv": np.ascontiguousarray(wvm).astype(bf16),
                "wp": np.ascontiguousarray(proj_w[64 * G * g : 64 * G * (g + 1), :]).astype(bf16),
            }
        )

    nc = _get_nc()
    res = run_bass_kernel_spmd(nc, in_maps, core_ids=list(range(NCORES)))
    _NC_CACHE["last_result"] = res

    bias_row = (br[2].reshape(DIM).astype(np.float64) @ proj_w.astype(np.float64)
                + proj_b.astype(np.float64)).astype(np.float32)
    out = np.zeros((B, N, DIM), np.float32)
    for b in range(B):
        acc = np.zeros((N, DIM), np.float64)
        for g in range(4):
            acc += res.results[4 * b + g]["out"].astype(np.float64)
        out[b] = (-acc).astype(np.float32) + bias_row
    return out
